# revision 60
# baseline (speedup 1.0000x reference)
"""Trainium2 Bass kernel for CustomMultiheadAttention.

Problem shapes: query/key/value [2048, 4, 1024] f32, causal mask [2048, 2048],
Wq/Wk/Wv/Wo [1024, 1024] (torch Linear layout [out, in]), biases [1024].
16 heads, head dim 64.

Sharding over 8 cores: core c -> (batch b = c // 2, head-group hg = c % 2).
Each core computes 8 heads (an E-slice of 512 rows of Wq/Wk/Wv, 512 cols of
Wo) for one batch. Host sums the two partial output projections per batch and
adds bo.

Device algorithm per core (all matmul inputs bf16, f32 PSUM accumulation):
  1. Q^T/K^T projections ko-outer across 8 PSUM banks so matmuls start on the
     first arriving 256KB x-chunk; x chunks stream on the two HWDGE rings
     (sync/scalar) while weights ride the gpsimd SWDGE path.
  2. Attention processes HEAD PAIRS (partitions 0-63 / 64-127 of qt/kt): the
     two scores matmuls (K=64 each) are emitted back-to-back so the PE runs
     them concurrently in disjoint row-group halves of the array
     (tile_position auto-derived from base partitions). One ACT exp
     instruction covers both heads' scores tile [128, 2, 512]. V carries a
     ones column per head so PV (K=128) accumulates [65, 512] = [num^T; den].
     Normalize: reciprocal_approx_fast on DVE (PSUM row 64) -> gpsimd
     partition_broadcast -> DVE multiply into attnT bf16.
  3. V projection for s-blocks 4-15 and the output projection
     out_part[t, f] = attnT.T @ Wo_slice^T are deferred "filler" tiles popped
     one per attention iteration to keep the PE dense (HAM stays warm).
"""

import math
import os
import sys

import numpy as np

for _p in ("/opt/trn_rl_repo", os.path.expanduser("~/.axon_site/_ro/trn_rl_repo")):
    if os.path.isdir(_p) and _p not in sys.path:
        sys.path.insert(0, _p)

import ml_dtypes  # noqa: E402

import concourse.bass as bass  # noqa: E402
import concourse.tile as tile  # noqa: E402
from concourse import bacc, bass_utils, library_config, mybir  # noqa: E402

# Problem constants
T, S, B, E, H = 2048, 2048, 4, 1024, 16
D = E // H  # 64
NCORES = 8
HC = H // 2  # heads per core
EH = HC * D  # 512 per-core E-slice
P = 128
TC = 512  # t-chunk
NT = T // TC  # 4
NSB = S // P  # 16 s-blocks
KO = E // P  # 8 contraction chunks for projections
KHD = EH // P  # 4 contraction chunks for out proj
NPAIR = HC // 2  # 4 head pairs per core
VW = D + 1  # 65: head V width incl ones column
NVA = 4  # V-proj s-blocks done in phase 1; rest become fillers
BF16 = mybir.dt.bfloat16
F32 = mybir.dt.float32
NPBF16 = ml_dtypes.bfloat16

_CACHE: dict = {}


def _build_nc():
    nc = bacc.Bacc(
        "TRN2",
        target_bir_lowering=False,
        debug=False,
        enable_asserts=True,
        num_devices=NCORES,
    )
    AF = mybir.ActivationFunctionType

    xq_t = nc.dram_tensor("xq_t", [E, T], BF16, kind="ExternalInput").ap()
    xk_t = nc.dram_tensor("xk_t", [E, T], BF16, kind="ExternalInput").ap()
    xv_t = nc.dram_tensor("xv_t", [E, T], BF16, kind="ExternalInput").ap()
    wq_t = nc.dram_tensor("wq_t", [E, EH], BF16, kind="ExternalInput").ap()
    wk_t = nc.dram_tensor("wk_t", [E, EH], BF16, kind="ExternalInput").ap()
    wv_t = nc.dram_tensor("wv_t", [E, EH], BF16, kind="ExternalInput").ap()
    wo_t = nc.dram_tensor("wo_t", [EH, E], BF16, kind="ExternalInput").ap()
    bq_d = nc.dram_tensor("bq_d", [P, KHD], F32, kind="ExternalInput").ap()
    bk_d = nc.dram_tensor("bk_d", [P, KHD], F32, kind="ExternalInput").ap()
    bv_d = nc.dram_tensor("bv_d", [P, EH], F32, kind="ExternalInput").ap()
    mask_d = nc.dram_tensor("mask_d", [P, 4, P], BF16, kind="ExternalInput").ap()
    out_p = nc.dram_tensor("out_part", [T, E], BF16, kind="ExternalOutput").ap()

    from contextlib import ExitStack

    with tile.TileContext(nc) as tc, ExitStack() as ctx:
        persist = ctx.enter_context(tc.tile_pool(name="persist", bufs=1))
        xpool = ctx.enter_context(tc.tile_pool(name="x", bufs=2))
        exps = ctx.enter_context(tc.tile_pool(name="exps", bufs=4))
        evac = ctx.enter_context(tc.tile_pool(name="evac", bufs=2))
        otpool = ctx.enter_context(tc.tile_pool(name="ot", bufs=8))

        nc.gpsimd.load_library(library_config.attn)  # for partition_broadcast

        # ---- persistent SBUF tensors
        wq_sb = persist.tile([P, KO, EH], BF16, tag="wq")
        wk_sb = persist.tile([P, KO, EH], BF16, tag="wk")
        wv_sb = persist.tile([P, KO, EH], BF16, tag="wv")
        wo_sb = persist.tile([P, KHD, E], BF16, tag="wo")
        bq_sb = persist.tile([P, KHD], F32, tag="bq")
        bk_sb = persist.tile([P, KHD], F32, tag="bk")
        bv_sb = persist.tile([P, EH], F32, tag="bv")
        mask_sb = persist.tile([P, 4, P], BF16, tag="mask")
        qt_sb = persist.tile([P, KHD, T], BF16, tag="qt")
        kt_sb = persist.tile([P, KHD, T], BF16, tag="kt")
        # Split per s-block / per tj so the tile dependency tracker never
        # creates false write->read ordering across unrelated regions
        # (out-proj reads of an old tj must not wait on the current
        # pair's norm multiplies, nor pv reads on later v-proj writes).
        v_sbs = [
            persist.tile([P, HC, P], BF16, tag=f"v{sb}", name=f"v_sb{sb}")
            for sb in range(NSB)
        ]
        attnTs = [
            persist.tile([P, KHD, TC], BF16, tag=f"attnT{j}", name=f"attnT{j}")
            for j in range(NT)
        ]

        # wq/bq go first on the scalar HWDGE ring (needed by the first MMs;
        # the gpsimd SWDGE path starts too late behind the library load).
        # Everything needed later rides SWDGE (gpsimd) so the two HWDGE
        # rings (sync/scalar) are free for the big x streams. The xq odd-ko
        # chunks interleave between wq chunks so the ko-outer projection
        # loop never waits ~1MB of weight traffic for its next x chunk.
        xq_sb = xpool.tile([P, KO, T], BF16, tag="xt")
        wq_src = wq_t.rearrange("(ko p) m -> p ko m", p=P)
        xq_src = xq_t.rearrange("(ko p) t -> p ko t", p=P)
        h0 = slice(0, T // 2)
        # first wq chunk split in db halves so the first matmuls start
        # on 64KB instead of 128KB
        nc.scalar.dma_start(wq_sb[:, 0, 0 : 2 * P], wq_src[:, 0, 0 : 2 * P])
        nc.scalar.dma_start(wq_sb[:, 0, 2 * P : EH], wq_src[:, 0, 2 * P : EH])
        scalar_seq = [("w", 1), ("x", 1), ("w", 2), ("x", 3),
                      ("w", 3), ("x", 5), ("w", 4), ("x", 7), ("w", 5),
                      ("w", 6), ("w", 7)]
        for kind, ko in scalar_seq:
            if kind == "w":
                nc.scalar.dma_start(wq_sb[:, ko, :], wq_src[:, ko, :])
            else:
                nc.scalar.dma_start(xq_sb[:, ko, h0], xq_src[:, ko, h0])
        nc.scalar.dma_start(bq_sb[:], bq_d)
        wk_src = wk_t.rearrange("(ko p) m -> p ko m", p=P)
        for ko in range(KO):
            nc.gpsimd.dma_start(wk_sb[:, ko, :], wk_src[:, ko, :])
        nc.gpsimd.dma_start(bk_sb[:], bk_d)
        wv_src = wv_t.rearrange("(ko p) m -> p ko m", p=P)
        for ko in range(KO):
            nc.gpsimd.dma_start(wv_sb[:, ko, :], wv_src[:, ko, :])
        nc.gpsimd.dma_start(bv_sb[:], bv_d)
        nc.gpsimd.dma_start(mask_sb[:], mask_d)
        wo_src = wo_t.rearrange("(ko p) m -> p ko m", p=P)
        for ko in range(KHD):
            nc.gpsimd.dma_start(wo_sb[:, ko, :], wo_src[:, ko, :])

        # ones columns for the softmax denominator rows of V; the padding
        # columns D+1..127 are zeroed once so PV rows 65-127 accumulate zeros.
        for sb in range(NSB):
            nc.vector.memset(v_sbs[sb][:, :, D : P], 0.0)
            nc.vector.memset(v_sbs[sb][:, :, D : D + 1], 1.0)

        # x streams: [128, 1024] half-chunks, even ko on sync ring, odd on
        # scalar ring, h-major so arrival order matches the ko-outer passes.
        def load_x(x_sb, x_dram, skip_h0_odd=False):
            x_src = x_dram.rearrange("(ko p) t -> p ko t", p=P)
            for h in range(2):
                cols = slice(h * (T // 2), (h + 1) * (T // 2))
                for ko in range(KO):
                    if skip_h0_odd and h == 0 and ko % 2 == 1:
                        continue  # already queued interleaved with wq above
                    eng = nc.sync if ko % 2 == 0 else nc.scalar
                    eng.dma_start(x_sb[:, ko, cols], x_src[:, ko, cols])

        load_x(xq_sb, xq_t, skip_h0_odd=True)
        xk_sb = xpool.tile([P, KO, T], BF16, tag="xt")
        load_x(xk_sb, xk_t)

        # ---- phase 1: Q/K projections (ko-outer, 8 PSUM banks) + V part A
        with tc.tile_pool(name="psA", bufs=8, space="PSUM") as psA:
            for x_sb, w_sb, b_sb, dst in (
                (xq_sb, wq_sb, bq_sb, qt_sb),
                (xk_sb, wk_sb, bk_sb, kt_sb),
            ):
                for h in range(2):
                    ps = {}
                    for db in range(KHD):
                        for tjj in range(2):
                            ps[(db, tjj)] = psA.tile(
                                [P, TC], F32, tag="pp", name=f"pp{db}{tjj}"
                            )
                    for ko in range(KO):
                        for db in range(KHD):
                            for tjj in range(2):
                                tj = 2 * h + tjj
                                nc.tensor.matmul(
                                    ps[(db, tjj)][:],
                                    lhsT=w_sb[:, ko, db * P : (db + 1) * P],
                                    rhs=x_sb[:, ko, tj * TC : (tj + 1) * TC],
                                    start=(ko == 0),
                                    stop=(ko == KO - 1),
                                )
                    for db in range(KHD):
                        for tjj in range(2):
                            tj = 2 * h + tjj
                            nc.vector.tensor_scalar_add(
                                dst[:, db, tj * TC : (tj + 1) * TC],
                                ps[(db, tjj)][:],
                                b_sb[:, db : db + 1],
                            )

            # xv reuses xq's ring slot; its DMAs wait on the last q matmul.
            xv_sb = xpool.tile([P, KO, T], BF16, tag="xt")
            load_x(xv_sb, xv_t)

            def v_proj_block(pool, sb):
                psv = pool.tile([P, EH], F32, tag="po" if pool is not psA else "pp")
                for ko in range(KO):
                    nc.tensor.matmul(
                        psv[:],
                        lhsT=xv_sb[:, ko, sb * P : (sb + 1) * P],
                        rhs=wv_sb[:, ko, :],
                        start=(ko == 0),
                        stop=(ko == KO - 1),
                    )
                nc.vector.tensor_add(
                    v_sbs[sb][:, :, 0:D],
                    psv[:].rearrange("p (h x) -> p h x", h=HC),
                    bv_sb[:].rearrange("p (h x) -> p h x", h=HC),
                )

            for sb in range(NVA):
                v_proj_block(psA, sb)

        # ---- phase 2: attention (head pairs) + deferred fillers
        with tc.tile_pool(name="psC", bufs=2, space="PSUM") as psC:
            pool_sel = [psC]
            fillers = []  # deferred tile emitters, popped into the PE stream

            def emit_filler():
                if fillers:
                    fillers.pop(0)()

            for sb in range(NVA, NSB):
                fillers.append(lambda sb=sb: v_proj_block(pool_sel[0], sb))

            evac_alt = [0]
            dma_rr = [0]
            out_dma_engs = (nc.sync, nc.scalar, nc.sync, nc.gpsimd, nc.scalar, nc.sync)

            def make_outproj(tb, fj):
                def _emit():
                    po = pool_sel[0].tile([P, TC], F32, tag="po")
                    for ko in range(KHD):
                        nc.tensor.matmul(
                            po[:],
                            lhsT=attnTs[tb // 4][:, ko, (tb % 4) * P : (tb % 4 + 1) * P],
                            rhs=wo_sb[:, ko, fj * TC : (fj + 1) * TC],
                            start=(ko == 0),
                            stop=(ko == KHD - 1),
                        )
                    ot = otpool.tile([P, TC], BF16, tag="ot")
                    evac_alt[0] = (evac_alt[0] + 1) % 3
                    if evac_alt[0] != 0:  # 2:1 vector-heavy: ACT is the co-pole
                        nc.vector.tensor_copy(ot[:], po[:])
                    else:
                        nc.scalar.copy(ot[:], po[:])
                    eng = out_dma_engs[dma_rr[0] % 6]
                    dma_rr[0] += 1
                    eng.dma_start(
                        out_p[tb * P : (tb + 1) * P, fj * TC : (fj + 1) * TC], ot[:]
                    )

                return _emit

            def start_pair(tj, ch):
                # pvs alloc is lazy (ensure_pvs): with psPV bufs=2 the new
                # tiles recycle the previous pair's buffers, so allocation
                # must happen after that pair's last pv writes even though
                # this pair's sc/exp work is peeled in earlier.
                return {
                    "tj": tj, "ch": ch, "ns": 4 * tj + 4,
                    "tcols": slice(tj * TC, (tj + 1) * TC),
                    "pvs": None, "ets": {}, "scs": {}, "norm": {}, "g0": 0,
                }

            def ensure_pvs(st):
                if st["pvs"] is None:
                    st["pvs"] = [
                        psPV.tile(
                            [P, TC], F32, tag="pv",
                            name=f"pv{st['tj']}{st['ch']}{u}",
                        )
                        for u in range(2)
                    ]

            def emit_sc(st, g):
                tj, ch = st["tj"], st["ch"]
                k = g - 4 * tj
                lo = P * k if k > 0 else 0  # cols < lo are fully masked
                sc2 = psS.tile([P, 2, TC], F32, tag="sc", name=f"sc{tj}{ch}{g}")
                for u in range(2):
                    pb = D * u
                    nc.tensor.matmul(
                        sc2[:, u, lo:TC],
                        lhsT=kt_sb[pb : pb + D, ch, g * P : (g + 1) * P],
                        rhs=qt_sb[pb : pb + D, ch, tj * TC + lo : (tj + 1) * TC],
                        start=True,
                        stop=True,
                    )
                st["scs"][g] = sc2

            def emit_exp(st, g):
                tj = st["tj"]
                sc2 = st["scs"].pop(g)
                et2 = exps.tile([P, 2, TC], BF16, tag="et", name=f"et{st['tj']}{st['ch']}{g}")
                k = g - 4 * tj
                if k >= 0:  # diagonal block: partial-span exp + boundary mask
                    nc.scalar.activation(
                        et2[:, :, P * k : TC],
                        sc2[:, :, P * k : TC],
                        AF.Exp,
                        scale=1.0 / math.sqrt(D),
                    )
                    for u in range(2):
                        nc.vector.tensor_mul(
                            et2[:, u, P * k : P * (k + 1)],
                            et2[:, u, P * k : P * (k + 1)],
                            mask_sb[:, k, :],
                        )
                else:
                    nc.scalar.activation(
                        et2[:], sc2[:], AF.Exp, scale=1.0 / math.sqrt(D)
                    )
                st["ets"][g] = et2

            def emit_pv_u(st, g, u, pop):
                ensure_pvs(st)
                et = st["ets"].pop(g) if pop else st["ets"][g]
                hh = 2 * st["ch"] + u
                k = g - 4 * st["tj"]
                lo = P * k if k > 0 else 0  # skip the all-zero masked prefix
                nc.tensor.matmul(
                    st["pvs"][u][:, lo:TC],
                    lhsT=v_sbs[g][:, hh, :],
                    rhs=et[:, u, lo:TC],
                    start=(g == 0),
                    stop=(g == st["ns"] - 1),
                )

            def emit_pv(st, g):
                emit_pv_u(st, g, 0, False)
                emit_pv_u(st, g, 1, True)

            def norm_u(st, u):
                ch, pvs = st["ch"], st["pvs"]
                # Evacuate [num^T; den] to SBUF in one fast copy so the PSUM
                # bank recycles immediately; the recip/broadcast/multiply
                # chain then runs off the PE-critical path.
                pvt = evac.tile([VW, TC], F32, tag="pvt")
                nc.vector.tensor_copy(pvt[:], pvs[u][0:VW, :])
                # plain copy moves the den row to partition 0 first: the
                # custom recip uop needs matching src/dst base partitions.
                rec = evac.tile([1, TC], F32, tag="rec")
                nc.vector.tensor_copy(rec[:], pvt[D : D + 1, :])
                nc.vector.reciprocal_approx_fast(rec[:], rec[:])
                rbs = evac.tile([D, TC], F32, tag="rbs")
                nc.gpsimd.partition_broadcast(rbs[:], rec[:])
                nc.vector.tensor_mul(
                    attnTs[st["tj"]][D * u : D * (u + 1), ch, :],
                    pvt[0:D, :],
                    rbs[:],
                )

            pair_list = [(tj, ch) for tj in range(NT) for ch in range(NPAIR)]
            nxt = None
            ctx2 = ExitStack()
            psS = ctx2.enter_context(tc.tile_pool(name="psS", bufs=2, space="PSUM"))
            psPV = ctx2.enter_context(tc.tile_pool(name="psPV", bufs=2, space="PSUM"))
            for idx, (tj, ch) in enumerate(pair_list):
                st = nxt if nxt is not None else start_pair(tj, ch)
                nxt = None
                ns = st["ns"]
                # 2-g batches: [sc, sc | exp, exp | pv x4 | filler] so the
                # PE sees long same-shape matmul runs (fewer weight-shape
                # transitions, which measurably stretch the stream).
                for gg in range(st["g0"], ns, 2):
                    emit_sc(st, gg)
                    emit_sc(st, gg + 1)
                    emit_exp(st, gg)
                    emit_exp(st, gg + 1)
                    if gg >= 2:
                        emit_pv(st, gg - 2)  # PV runs two chunks behind scores
                        emit_pv(st, gg - 1)
                        emit_filler()
                # Peel the next pair's first sc/exp batch plus a burst of
                # fillers BEFORE this pair's tail pvs: the tail pvs need the
                # freshest two exps, so this gives ACT a head start while
                # the in-order PE queue stays fed.
                if idx + 1 < len(pair_list):
                    nxt = start_pair(*pair_list[idx + 1])
                    emit_sc(nxt, 0)
                    emit_sc(nxt, 1)
                    emit_exp(nxt, 0)
                    emit_exp(nxt, 1)
                    nxt["g0"] = 2
                for _ in range(4):
                    emit_filler()
                # Pair tail, u0 first so its pv-ring slot frees early.
                emit_pv_u(st, ns - 2, 0, False)
                emit_pv_u(st, ns - 1, 0, False)
                norm_u(st, 0)
                emit_pv_u(st, ns - 2, 1, True)
                emit_pv_u(st, ns - 1, 1, True)
                norm_u(st, 1)
                if ch == NPAIR - 1:
                    # queue this tj's out-proj tiles; they fill PE slots in tj+1
                    for tb in range(4 * tj, 4 * tj + 4):
                        for fj in range(E // TC):
                            fillers.append(make_outproj(tb, fj))
            ctx2.close()  # free the score/PV banks for a deep drain pool
            with tc.tile_pool(name="psD", bufs=6, space="PSUM") as psD:
                pool_sel[0] = psD
                while fillers:
                    emit_filler()

    nc.compile()
    return nc


def _get_nc():
    if "nc" not in _CACHE:
        _CACHE["nc"] = _build_nc()
    return _CACHE["nc"]


def _prep_in_maps(query, key, value, attn_mask, Wq, bq, Wk, bk, Wv, bv, Wo, bo):
    """Host-side prep: slices, transposes, bf16 casts. Returns in_maps[8]."""
    f32 = np.float32
    xt = {}  # (kind, b) -> [E, T] bf16
    for b in range(B):
        xt[("q", b)] = np.ascontiguousarray(query[:, b, :].T).astype(NPBF16)
        xt[("k", b)] = np.ascontiguousarray(key[:, b, :].T).astype(NPBF16)
        xt[("v", b)] = np.ascontiguousarray(value[:, b, :].T).astype(NPBF16)
    wt = {}
    for hg in range(2):
        sl = slice(EH * hg, EH * hg + EH)
        wt[("q", hg)] = np.ascontiguousarray(Wq[sl, :].T).astype(NPBF16)
        wt[("k", hg)] = np.ascontiguousarray(Wk[sl, :].T).astype(NPBF16)
        wt[("v", hg)] = np.ascontiguousarray(Wv[sl, :].T).astype(NPBF16)
        wt[("o", hg)] = np.ascontiguousarray(Wo[:, sl].T).astype(NPBF16)
        wt[("bq", hg)] = np.ascontiguousarray(
            bq[sl].astype(f32).reshape(KHD, P).T
        )
        wt[("bk", hg)] = np.ascontiguousarray(
            bk[sl].astype(f32).reshape(KHD, P).T
        )
        wt[("bv", hg)] = np.ascontiguousarray(
            np.tile(bv[sl].astype(f32)[None, :], (P, 1))
        )
    # mask patterns: for a scores tile with s0 = t0 + 128*o, pattern
    # [p, o, f] = 0 if attn_mask[t0+f, s0+p] (masked) else 1.
    t0 = 512
    patts = []
    for o in range(4):
        s0 = t0 + P * o
        full = (~np.asarray(attn_mask[t0 : t0 + TC, s0 : s0 + P])).T.astype(NPBF16)
        patts.append(full[:, P * o : P * (o + 1)])
    mask_tiles = np.ascontiguousarray(np.stack(patts, axis=1))  # [P, 4, 128]

    in_maps = []
    for c in range(NCORES):
        b, hg = c // 2, c % 2
        in_maps.append(
            {
                "xq_t": xt[("q", b)],
                "xk_t": xt[("k", b)],
                "xv_t": xt[("v", b)],
                "wq_t": wt[("q", hg)],
                "wk_t": wt[("k", hg)],
                "wv_t": wt[("v", hg)],
                "wo_t": wt[("o", hg)],
                "bq_d": wt[("bq", hg)],
                "bk_d": wt[("bk", hg)],
                "bv_d": wt[("bv", hg)],
                "mask_d": mask_tiles,
            }
        )
    return in_maps


def _run_on_hw(in_maps, trace=False, **kwargs):
    nc = _get_nc()
    return bass_utils.run_bass_kernel_spmd(
        nc, in_maps, core_ids=list(range(NCORES)), trace=trace, **kwargs
    )


def _gather(results, bo):
    outs = []
    for b in range(B):
        part = np.asarray(results[2 * b]["out_part"], dtype=np.float32) + np.asarray(
            results[2 * b + 1]["out_part"], dtype=np.float32
        )
        outs.append(part)
    out = np.stack(outs, axis=1)  # [T, B, E]
    out += np.asarray(bo, dtype=np.float32)[None, None, :]
    return out.astype(np.float32)


def _numpy_fallback(query, key, value, attn_mask, Wq, bq, Wk, bk, Wv, bv, Wo, bo):
    """Exact f32 numpy replication of the reference (for non-causal masks)."""
    f32 = np.float32
    query, key, value = (np.asarray(a, f32) for a in (query, key, value))
    q = (np.einsum("tbe,fe->btf", query, Wq, dtype=f32) + bq).reshape(B, T, H, D)
    k = (np.einsum("sbe,fe->bsf", key, Wk, dtype=f32) + bk).reshape(B, S, H, D)
    v = (np.einsum("sbe,fe->bsf", value, Wv, dtype=f32) + bv).reshape(B, S, H, D)
    q, k, v = (a.transpose(0, 2, 1, 3) for a in (q, k, v))
    out = np.empty((B, H, T, D), f32)
    mask = np.asarray(attn_mask)
    for b in range(B):
        for h in range(H):
            sc = (q[b, h] @ k[b, h].T) / np.float32(math.sqrt(D))
            sc = np.where(mask, -np.inf, sc)
            m = np.max(sc, axis=-1, keepdims=True)
            m = np.where(np.isfinite(m), m, 0.0)
            e = np.exp(sc - m)
            p = e / np.sum(e, axis=-1, keepdims=True)
            p = np.where(np.isinf(sc), 0.0, p)
            out[b, h] = p @ v[b, h]
    out = out.transpose(0, 2, 1, 3).reshape(B, T, E)
    out = out @ np.asarray(Wo, f32).T + bo
    return np.ascontiguousarray(out.transpose(1, 0, 2)).astype(f32)


def kernel(query, key, value, attn_mask, Wq, bq, Wk, bk, Wv, bv, Wo, bo):
    mask = np.asarray(attn_mask)
    causal = mask.shape == (T, S) and np.array_equal(
        mask, np.triu(np.ones((T, S), dtype=bool), k=1)
    )
    if not causal:
        return _numpy_fallback(
            query, key, value, attn_mask, Wq, bq, Wk, bk, Wv, bv, Wo, bo
        )
    in_maps = _prep_in_maps(
        query, key, value, attn_mask, Wq, bq, Wk, bk, Wv, bv, Wo, bo
    )
    res = _run_on_hw(in_maps)
    return _gather(res.results, bo)



# revision 61
# speedup vs baseline: 1.0027x; 1.0027x over previous
"""Trainium2 Bass kernel for CustomMultiheadAttention.

Problem shapes: query/key/value [2048, 4, 1024] f32, causal mask [2048, 2048],
Wq/Wk/Wv/Wo [1024, 1024] (torch Linear layout [out, in]), biases [1024].
16 heads, head dim 64.

Sharding over 8 cores: core c -> (batch b = c // 2, head-group hg = c % 2).
Each core computes 8 heads (an E-slice of 512 rows of Wq/Wk/Wv, 512 cols of
Wo) for one batch. Host sums the two partial output projections per batch and
adds bo.

Device algorithm per core (all matmul inputs bf16, f32 PSUM accumulation):
  1. Q^T/K^T projections ko-outer across 8 PSUM banks so matmuls start on the
     first arriving 256KB x-chunk; x chunks stream on the two HWDGE rings
     (sync/scalar) while weights ride the gpsimd SWDGE path.
  2. Attention processes HEAD PAIRS (partitions 0-63 / 64-127 of qt/kt): the
     two scores matmuls (K=64 each) are emitted back-to-back so the PE runs
     them concurrently in disjoint row-group halves of the array
     (tile_position auto-derived from base partitions). One ACT exp
     instruction covers both heads' scores tile [128, 2, 512]. V carries a
     ones column per head so PV (K=128) accumulates [65, 512] = [num^T; den].
     Normalize: reciprocal_approx_fast on DVE (PSUM row 64) -> gpsimd
     partition_broadcast -> DVE multiply into attnT bf16.
  3. V projection for s-blocks 4-15 and the output projection
     out_part[t, f] = attnT.T @ Wo_slice^T are deferred "filler" tiles popped
     one per attention iteration to keep the PE dense (HAM stays warm).
"""

import math
import os
import sys

import numpy as np

for _p in ("/opt/trn_rl_repo", os.path.expanduser("~/.axon_site/_ro/trn_rl_repo")):
    if os.path.isdir(_p) and _p not in sys.path:
        sys.path.insert(0, _p)

import ml_dtypes  # noqa: E402

import concourse.bass as bass  # noqa: E402
import concourse.tile as tile  # noqa: E402
from concourse import bacc, bass_utils, library_config, mybir  # noqa: E402

# Problem constants
T, S, B, E, H = 2048, 2048, 4, 1024, 16
D = E // H  # 64
NCORES = 8
HC = H // 2  # heads per core
EH = HC * D  # 512 per-core E-slice
P = 128
TC = 512  # t-chunk
NT = T // TC  # 4
NSB = S // P  # 16 s-blocks
KO = E // P  # 8 contraction chunks for projections
KHD = EH // P  # 4 contraction chunks for out proj
NPAIR = HC // 2  # 4 head pairs per core
VW = D + 1  # 65: head V width incl ones column
NVA = 4  # V-proj s-blocks done in phase 1; rest become fillers
BF16 = mybir.dt.bfloat16
F32 = mybir.dt.float32
NPBF16 = ml_dtypes.bfloat16

_CACHE: dict = {}


def _build_nc():
    nc = bacc.Bacc(
        "TRN2",
        target_bir_lowering=False,
        debug=False,
        enable_asserts=True,
        num_devices=NCORES,
    )
    AF = mybir.ActivationFunctionType

    xq_t = nc.dram_tensor("xq_t", [E, T], BF16, kind="ExternalInput").ap()
    xk_t = nc.dram_tensor("xk_t", [E, T], BF16, kind="ExternalInput").ap()
    xv_t = nc.dram_tensor("xv_t", [E, T], BF16, kind="ExternalInput").ap()
    wq_t = nc.dram_tensor("wq_t", [E, EH], BF16, kind="ExternalInput").ap()
    wk_t = nc.dram_tensor("wk_t", [E, EH], BF16, kind="ExternalInput").ap()
    wv_t = nc.dram_tensor("wv_t", [E, EH], BF16, kind="ExternalInput").ap()
    wo_t = nc.dram_tensor("wo_t", [EH, E], BF16, kind="ExternalInput").ap()
    bq_d = nc.dram_tensor("bq_d", [P, KHD], F32, kind="ExternalInput").ap()
    bk_d = nc.dram_tensor("bk_d", [P, KHD], F32, kind="ExternalInput").ap()
    bv_d = nc.dram_tensor("bv_d", [P, EH], F32, kind="ExternalInput").ap()
    mask_d = nc.dram_tensor("mask_d", [P, 4, P], BF16, kind="ExternalInput").ap()
    out_p = nc.dram_tensor("out_part", [T, E], BF16, kind="ExternalOutput").ap()

    from contextlib import ExitStack

    with tile.TileContext(nc) as tc, ExitStack() as ctx:
        persist = ctx.enter_context(tc.tile_pool(name="persist", bufs=1))
        xpool = ctx.enter_context(tc.tile_pool(name="x", bufs=2))
        exps = ctx.enter_context(tc.tile_pool(name="exps", bufs=4))
        evac = ctx.enter_context(tc.tile_pool(name="evac", bufs=2))
        otpool = ctx.enter_context(tc.tile_pool(name="ot", bufs=8))

        nc.gpsimd.load_library(library_config.attn)  # for partition_broadcast

        # ---- persistent SBUF tensors
        wq_sb = persist.tile([P, KO, EH], BF16, tag="wq")
        wk_sb = persist.tile([P, KO, EH], BF16, tag="wk")
        wv_sb = persist.tile([P, KO, EH], BF16, tag="wv")
        wo_sb = persist.tile([P, KHD, E], BF16, tag="wo")
        bq_sb = persist.tile([P, KHD], F32, tag="bq")
        bk_sb = persist.tile([P, KHD], F32, tag="bk")
        bv_sb = persist.tile([P, EH], F32, tag="bv")
        mask_sb = persist.tile([P, 4, P], BF16, tag="mask")
        qt_sb = persist.tile([P, KHD, T], BF16, tag="qt")
        kt_sb = persist.tile([P, KHD, T], BF16, tag="kt")
        # Split per s-block / per tj so the tile dependency tracker never
        # creates false write->read ordering across unrelated regions
        # (out-proj reads of an old tj must not wait on the current
        # pair's norm multiplies, nor pv reads on later v-proj writes).
        v_sbs = [
            persist.tile([P, HC, P], BF16, tag=f"v{sb}", name=f"v_sb{sb}")
            for sb in range(NSB)
        ]
        attnTs = [
            persist.tile([P, KHD, TC], BF16, tag=f"attnT{j}", name=f"attnT{j}")
            for j in range(NT)
        ]

        # wq/bq go first on the scalar HWDGE ring (needed by the first MMs;
        # the gpsimd SWDGE path starts too late behind the library load).
        # Everything needed later rides SWDGE (gpsimd) so the two HWDGE
        # rings (sync/scalar) are free for the big x streams. The xq odd-ko
        # chunks interleave between wq chunks so the ko-outer projection
        # loop never waits ~1MB of weight traffic for its next x chunk.
        xq_sb = xpool.tile([P, KO, T], BF16, tag="xt")
        wq_src = wq_t.rearrange("(ko p) m -> p ko m", p=P)
        xq_src = xq_t.rearrange("(ko p) t -> p ko t", p=P)
        h0 = slice(0, T // 2)
        scalar_seq = [("w", 0), ("w", 1), ("x", 1), ("w", 2), ("x", 3),
                      ("w", 3), ("x", 5), ("w", 4), ("x", 7), ("w", 5),
                      ("w", 6), ("w", 7)]
        for kind, ko in scalar_seq:
            if kind == "w":
                nc.scalar.dma_start(wq_sb[:, ko, :], wq_src[:, ko, :])
            else:
                nc.scalar.dma_start(xq_sb[:, ko, h0], xq_src[:, ko, h0])
        nc.scalar.dma_start(bq_sb[:], bq_d)
        wk_src = wk_t.rearrange("(ko p) m -> p ko m", p=P)
        for ko in range(KO):
            nc.gpsimd.dma_start(wk_sb[:, ko, :], wk_src[:, ko, :])
        nc.gpsimd.dma_start(bk_sb[:], bk_d)
        wv_src = wv_t.rearrange("(ko p) m -> p ko m", p=P)
        for ko in range(KO):
            nc.gpsimd.dma_start(wv_sb[:, ko, :], wv_src[:, ko, :])
        nc.gpsimd.dma_start(bv_sb[:], bv_d)
        nc.gpsimd.dma_start(mask_sb[:], mask_d)
        wo_src = wo_t.rearrange("(ko p) m -> p ko m", p=P)
        for ko in range(KHD):
            nc.gpsimd.dma_start(wo_sb[:, ko, :], wo_src[:, ko, :])

        # ones columns for the softmax denominator rows of V; the padding
        # columns D+1..127 are zeroed once so PV rows 65-127 accumulate zeros.
        for sb in range(NSB):
            nc.vector.memset(v_sbs[sb][:, :, D : P], 0.0)
            nc.vector.memset(v_sbs[sb][:, :, D : D + 1], 1.0)

        # x streams: [128, 1024] half-chunks, even ko on sync ring, odd on
        # scalar ring, h-major so arrival order matches the ko-outer passes.
        def load_x(x_sb, x_dram, skip_h0_odd=False):
            x_src = x_dram.rearrange("(ko p) t -> p ko t", p=P)
            for h in range(2):
                cols = slice(h * (T // 2), (h + 1) * (T // 2))
                for ko in range(KO):
                    if skip_h0_odd and h == 0 and ko % 2 == 1:
                        continue  # already queued interleaved with wq above
                    eng = nc.sync if ko % 2 == 0 else nc.scalar
                    eng.dma_start(x_sb[:, ko, cols], x_src[:, ko, cols])

        load_x(xq_sb, xq_t, skip_h0_odd=True)
        xk_sb = xpool.tile([P, KO, T], BF16, tag="xt")
        load_x(xk_sb, xk_t)

        # ---- phase 1: Q/K projections (ko-outer, 8 PSUM banks) + V part A
        with tc.tile_pool(name="psA", bufs=8, space="PSUM") as psA:
            for x_sb, w_sb, b_sb, dst in (
                (xq_sb, wq_sb, bq_sb, qt_sb),
                (xk_sb, wk_sb, bk_sb, kt_sb),
            ):
                for h in range(2):
                    ps = {}
                    for db in range(KHD):
                        for tjj in range(2):
                            ps[(db, tjj)] = psA.tile(
                                [P, TC], F32, tag="pp", name=f"pp{db}{tjj}"
                            )
                    for ko in range(KO):
                        for db in range(KHD):
                            for tjj in range(2):
                                tj = 2 * h + tjj
                                nc.tensor.matmul(
                                    ps[(db, tjj)][:],
                                    lhsT=w_sb[:, ko, db * P : (db + 1) * P],
                                    rhs=x_sb[:, ko, tj * TC : (tj + 1) * TC],
                                    start=(ko == 0),
                                    stop=(ko == KO - 1),
                                )
                    for db in range(KHD):
                        for tjj in range(2):
                            tj = 2 * h + tjj
                            nc.vector.tensor_scalar_add(
                                dst[:, db, tj * TC : (tj + 1) * TC],
                                ps[(db, tjj)][:],
                                b_sb[:, db : db + 1],
                            )

            # xv reuses xq's ring slot; its DMAs wait on the last q matmul.
            xv_sb = xpool.tile([P, KO, T], BF16, tag="xt")
            load_x(xv_sb, xv_t)

            def v_proj_block(pool, sb):
                psv = pool.tile([P, EH], F32, tag="po" if pool is not psA else "pp")
                for ko in range(KO):
                    nc.tensor.matmul(
                        psv[:],
                        lhsT=xv_sb[:, ko, sb * P : (sb + 1) * P],
                        rhs=wv_sb[:, ko, :],
                        start=(ko == 0),
                        stop=(ko == KO - 1),
                    )
                nc.vector.tensor_add(
                    v_sbs[sb][:, :, 0:D],
                    psv[:].rearrange("p (h x) -> p h x", h=HC),
                    bv_sb[:].rearrange("p (h x) -> p h x", h=HC),
                )

            for sb in range(NVA):
                v_proj_block(psA, sb)

        # ---- phase 2: attention (head pairs) + deferred fillers
        with tc.tile_pool(name="psC", bufs=2, space="PSUM") as psC:
            pool_sel = [psC]
            fillers = []  # deferred tile emitters, popped into the PE stream

            def emit_filler():
                if fillers:
                    fillers.pop(0)()

            for sb in range(NVA, NSB):
                fillers.append(lambda sb=sb: v_proj_block(pool_sel[0], sb))

            evac_alt = [0]
            dma_rr = [0]
            out_dma_engs = (nc.sync, nc.scalar, nc.sync, nc.gpsimd, nc.scalar, nc.sync)

            def make_outproj(tb, fj):
                def _emit():
                    po = pool_sel[0].tile([P, TC], F32, tag="po")
                    for ko in range(KHD):
                        nc.tensor.matmul(
                            po[:],
                            lhsT=attnTs[tb // 4][:, ko, (tb % 4) * P : (tb % 4 + 1) * P],
                            rhs=wo_sb[:, ko, fj * TC : (fj + 1) * TC],
                            start=(ko == 0),
                            stop=(ko == KHD - 1),
                        )
                    ot = otpool.tile([P, TC], BF16, tag="ot")
                    evac_alt[0] = (evac_alt[0] + 1) % 3
                    if evac_alt[0] != 0:  # 2:1 vector-heavy: ACT is the co-pole
                        nc.vector.tensor_copy(ot[:], po[:])
                    else:
                        nc.scalar.copy(ot[:], po[:])
                    eng = out_dma_engs[dma_rr[0] % 6]
                    dma_rr[0] += 1
                    eng.dma_start(
                        out_p[tb * P : (tb + 1) * P, fj * TC : (fj + 1) * TC], ot[:]
                    )

                return _emit

            def start_pair(tj, ch):
                # pvs alloc is lazy (ensure_pvs): with psPV bufs=2 the new
                # tiles recycle the previous pair's buffers, so allocation
                # must happen after that pair's last pv writes even though
                # this pair's sc/exp work is peeled in earlier.
                return {
                    "tj": tj, "ch": ch, "ns": 4 * tj + 4,
                    "tcols": slice(tj * TC, (tj + 1) * TC),
                    "pvs": None, "ets": {}, "scs": {}, "norm": {}, "g0": 0,
                }

            def ensure_pvs(st):
                if st["pvs"] is None:
                    st["pvs"] = [
                        psPV.tile(
                            [P, TC], F32, tag="pv",
                            name=f"pv{st['tj']}{st['ch']}{u}",
                        )
                        for u in range(2)
                    ]

            def emit_sc(st, g):
                tj, ch = st["tj"], st["ch"]
                k = g - 4 * tj
                lo = P * k if k > 0 else 0  # cols < lo are fully masked
                sc2 = psS.tile([P, 2, TC], F32, tag="sc", name=f"sc{tj}{ch}{g}")
                for u in range(2):
                    pb = D * u
                    nc.tensor.matmul(
                        sc2[:, u, lo:TC],
                        lhsT=kt_sb[pb : pb + D, ch, g * P : (g + 1) * P],
                        rhs=qt_sb[pb : pb + D, ch, tj * TC + lo : (tj + 1) * TC],
                        start=True,
                        stop=True,
                    )
                st["scs"][g] = sc2

            def emit_exp(st, g):
                tj = st["tj"]
                sc2 = st["scs"].pop(g)
                et2 = exps.tile([P, 2, TC], BF16, tag="et", name=f"et{st['tj']}{st['ch']}{g}")
                k = g - 4 * tj
                if k >= 0:  # diagonal block: partial-span exp + boundary mask
                    nc.scalar.activation(
                        et2[:, :, P * k : TC],
                        sc2[:, :, P * k : TC],
                        AF.Exp,
                        scale=1.0 / math.sqrt(D),
                    )
                    for u in range(2):
                        nc.vector.tensor_mul(
                            et2[:, u, P * k : P * (k + 1)],
                            et2[:, u, P * k : P * (k + 1)],
                            mask_sb[:, k, :],
                        )
                else:
                    nc.scalar.activation(
                        et2[:], sc2[:], AF.Exp, scale=1.0 / math.sqrt(D)
                    )
                st["ets"][g] = et2

            def emit_pv_u(st, g, u, pop):
                ensure_pvs(st)
                et = st["ets"].pop(g) if pop else st["ets"][g]
                hh = 2 * st["ch"] + u
                k = g - 4 * st["tj"]
                lo = P * k if k > 0 else 0  # skip the all-zero masked prefix
                nc.tensor.matmul(
                    st["pvs"][u][:, lo:TC],
                    lhsT=v_sbs[g][:, hh, :],
                    rhs=et[:, u, lo:TC],
                    start=(g == 0),
                    stop=(g == st["ns"] - 1),
                )

            def emit_pv(st, g):
                emit_pv_u(st, g, 0, False)
                emit_pv_u(st, g, 1, True)

            def norm_u(st, u):
                ch, pvs = st["ch"], st["pvs"]
                # Evacuate [num^T; den] to SBUF in one fast copy so the PSUM
                # bank recycles immediately; the recip/broadcast/multiply
                # chain then runs off the PE-critical path.
                pvt = evac.tile([VW, TC], F32, tag="pvt")
                nc.vector.tensor_copy(pvt[:], pvs[u][0:VW, :])
                # plain copy moves the den row to partition 0 first: the
                # custom recip uop needs matching src/dst base partitions.
                rec = evac.tile([1, TC], F32, tag="rec")
                nc.vector.tensor_copy(rec[:], pvt[D : D + 1, :])
                nc.vector.reciprocal_approx_fast(rec[:], rec[:])
                rbs = evac.tile([D, TC], F32, tag="rbs")
                nc.gpsimd.partition_broadcast(rbs[:], rec[:])
                nc.vector.tensor_mul(
                    attnTs[st["tj"]][D * u : D * (u + 1), ch, :],
                    pvt[0:D, :],
                    rbs[:],
                )

            pair_list = [(tj, ch) for tj in range(NT) for ch in range(NPAIR)]
            nxt = None
            ctx2 = ExitStack()
            psS = ctx2.enter_context(tc.tile_pool(name="psS", bufs=2, space="PSUM"))
            psPV = ctx2.enter_context(tc.tile_pool(name="psPV", bufs=2, space="PSUM"))
            for idx, (tj, ch) in enumerate(pair_list):
                st = nxt if nxt is not None else start_pair(tj, ch)
                nxt = None
                ns = st["ns"]
                # 2-g batches: [sc, sc | exp, exp | pv x4 | filler] so the
                # PE sees long same-shape matmul runs (fewer weight-shape
                # transitions, which measurably stretch the stream).
                for gg in range(st["g0"], ns, 2):
                    emit_sc(st, gg)
                    emit_sc(st, gg + 1)
                    emit_exp(st, gg)
                    emit_exp(st, gg + 1)
                    if gg >= 2:
                        emit_pv(st, gg - 2)  # PV runs two chunks behind scores
                        emit_pv(st, gg - 1)
                        emit_filler()
                # Peel the next pair's first sc/exp batch plus a burst of
                # fillers BEFORE this pair's tail pvs: the tail pvs need the
                # freshest two exps, so this gives ACT a head start while
                # the in-order PE queue stays fed.
                if idx + 1 < len(pair_list):
                    nxt = start_pair(*pair_list[idx + 1])
                    emit_sc(nxt, 0)
                    emit_sc(nxt, 1)
                    emit_exp(nxt, 0)
                    emit_exp(nxt, 1)
                    nxt["g0"] = 2
                for _ in range(3):
                    emit_filler()
                # Pair tail, u0 first so its pv-ring slot frees early.
                emit_pv_u(st, ns - 2, 0, False)
                emit_pv_u(st, ns - 1, 0, False)
                norm_u(st, 0)
                emit_pv_u(st, ns - 2, 1, True)
                emit_pv_u(st, ns - 1, 1, True)
                norm_u(st, 1)
                if ch == NPAIR - 1:
                    # queue this tj's out-proj tiles; they fill PE slots in tj+1
                    for tb in range(4 * tj, 4 * tj + 4):
                        for fj in range(E // TC):
                            fillers.append(make_outproj(tb, fj))
            ctx2.close()  # free the score/PV banks for a deep drain pool
            with tc.tile_pool(name="psD", bufs=6, space="PSUM") as psD:
                pool_sel[0] = psD
                while fillers:
                    emit_filler()

    nc.compile()
    return nc


def _get_nc():
    if "nc" not in _CACHE:
        _CACHE["nc"] = _build_nc()
    return _CACHE["nc"]


def _prep_in_maps(query, key, value, attn_mask, Wq, bq, Wk, bk, Wv, bv, Wo, bo):
    """Host-side prep: slices, transposes, bf16 casts. Returns in_maps[8]."""
    f32 = np.float32
    xt = {}  # (kind, b) -> [E, T] bf16
    for b in range(B):
        xt[("q", b)] = np.ascontiguousarray(query[:, b, :].T).astype(NPBF16)
        xt[("k", b)] = np.ascontiguousarray(key[:, b, :].T).astype(NPBF16)
        xt[("v", b)] = np.ascontiguousarray(value[:, b, :].T).astype(NPBF16)
    wt = {}
    for hg in range(2):
        sl = slice(EH * hg, EH * hg + EH)
        wt[("q", hg)] = np.ascontiguousarray(Wq[sl, :].T).astype(NPBF16)
        wt[("k", hg)] = np.ascontiguousarray(Wk[sl, :].T).astype(NPBF16)
        wt[("v", hg)] = np.ascontiguousarray(Wv[sl, :].T).astype(NPBF16)
        wt[("o", hg)] = np.ascontiguousarray(Wo[:, sl].T).astype(NPBF16)
        wt[("bq", hg)] = np.ascontiguousarray(
            bq[sl].astype(f32).reshape(KHD, P).T
        )
        wt[("bk", hg)] = np.ascontiguousarray(
            bk[sl].astype(f32).reshape(KHD, P).T
        )
        wt[("bv", hg)] = np.ascontiguousarray(
            np.tile(bv[sl].astype(f32)[None, :], (P, 1))
        )
    # mask patterns: for a scores tile with s0 = t0 + 128*o, pattern
    # [p, o, f] = 0 if attn_mask[t0+f, s0+p] (masked) else 1.
    t0 = 512
    patts = []
    for o in range(4):
        s0 = t0 + P * o
        full = (~np.asarray(attn_mask[t0 : t0 + TC, s0 : s0 + P])).T.astype(NPBF16)
        patts.append(full[:, P * o : P * (o + 1)])
    mask_tiles = np.ascontiguousarray(np.stack(patts, axis=1))  # [P, 4, 128]

    in_maps = []
    for c in range(NCORES):
        b, hg = c // 2, c % 2
        in_maps.append(
            {
                "xq_t": xt[("q", b)],
                "xk_t": xt[("k", b)],
                "xv_t": xt[("v", b)],
                "wq_t": wt[("q", hg)],
                "wk_t": wt[("k", hg)],
                "wv_t": wt[("v", hg)],
                "wo_t": wt[("o", hg)],
                "bq_d": wt[("bq", hg)],
                "bk_d": wt[("bk", hg)],
                "bv_d": wt[("bv", hg)],
                "mask_d": mask_tiles,
            }
        )
    return in_maps


def _run_on_hw(in_maps, trace=False, **kwargs):
    nc = _get_nc()
    return bass_utils.run_bass_kernel_spmd(
        nc, in_maps, core_ids=list(range(NCORES)), trace=trace, **kwargs
    )


def _gather(results, bo):
    outs = []
    for b in range(B):
        part = np.asarray(results[2 * b]["out_part"], dtype=np.float32) + np.asarray(
            results[2 * b + 1]["out_part"], dtype=np.float32
        )
        outs.append(part)
    out = np.stack(outs, axis=1)  # [T, B, E]
    out += np.asarray(bo, dtype=np.float32)[None, None, :]
    return out.astype(np.float32)


def _numpy_fallback(query, key, value, attn_mask, Wq, bq, Wk, bk, Wv, bv, Wo, bo):
    """Exact f32 numpy replication of the reference (for non-causal masks)."""
    f32 = np.float32
    query, key, value = (np.asarray(a, f32) for a in (query, key, value))
    q = (np.einsum("tbe,fe->btf", query, Wq, dtype=f32) + bq).reshape(B, T, H, D)
    k = (np.einsum("sbe,fe->bsf", key, Wk, dtype=f32) + bk).reshape(B, S, H, D)
    v = (np.einsum("sbe,fe->bsf", value, Wv, dtype=f32) + bv).reshape(B, S, H, D)
    q, k, v = (a.transpose(0, 2, 1, 3) for a in (q, k, v))
    out = np.empty((B, H, T, D), f32)
    mask = np.asarray(attn_mask)
    for b in range(B):
        for h in range(H):
            sc = (q[b, h] @ k[b, h].T) / np.float32(math.sqrt(D))
            sc = np.where(mask, -np.inf, sc)
            m = np.max(sc, axis=-1, keepdims=True)
            m = np.where(np.isfinite(m), m, 0.0)
            e = np.exp(sc - m)
            p = e / np.sum(e, axis=-1, keepdims=True)
            p = np.where(np.isinf(sc), 0.0, p)
            out[b, h] = p @ v[b, h]
    out = out.transpose(0, 2, 1, 3).reshape(B, T, E)
    out = out @ np.asarray(Wo, f32).T + bo
    return np.ascontiguousarray(out.transpose(1, 0, 2)).astype(f32)


def kernel(query, key, value, attn_mask, Wq, bq, Wk, bk, Wv, bv, Wo, bo):
    mask = np.asarray(attn_mask)
    causal = mask.shape == (T, S) and np.array_equal(
        mask, np.triu(np.ones((T, S), dtype=bool), k=1)
    )
    if not causal:
        return _numpy_fallback(
            query, key, value, attn_mask, Wq, bq, Wk, bk, Wv, bv, Wo, bo
        )
    in_maps = _prep_in_maps(
        query, key, value, attn_mask, Wq, bq, Wk, bk, Wv, bv, Wo, bo
    )
    res = _run_on_hw(in_maps)
    return _gather(res.results, bo)

